# revision 10
# baseline (speedup 1.0000x reference)
"""CostVolumeRight kernel for Trainium2 (8 NeuronCores, batch-parallel).

cost[b, d, y, x] = mean_c right[b,c,y,x] * left[b,c,y,x+d]  (d in [0,64), zero
beyond x+d >= W).

Strategy per core (1 batch element per core):
  For each image row y:
    - 3 band matmuls per C-half (fp32, exact): stationary = right columns
      (chunks of 128/128/64), moving = left columns covering the +63 band.
      PSUM row tile Q[128, 509]; Q[i, cbase + i + d] = C * cost[d, x].
    - DVE copy+scale (1/C) PSUM -> SBUF.
    - DMA SBUF -> DRAM scratch (row pitch 512).
    - Diagonal-strided DMA (stride 513 on the DRAM side, fully legal there)
      re-reads the band as aligned tiles G[i, (chunk,d)].
    - PE transpose -> F[(chunk,d), i]; ScalarE copy -> SBUF; DMA to out.
"""
import sys

sys.path.insert(0, "/opt/trn_rl_repo")

import numpy as np
from contextlib import ExitStack

import concourse.bass as bass
import concourse.tile as tile
from concourse import bacc, mybir
from concourse.bass_utils import run_bass_kernel_spmd

F32 = mybir.dt.float32

B, C, H, W, D = 8, 256, 160, 320, 64
P = 512          # qs staging row pitch
QP = 288         # compact DRAM scratch row pitch (3 windows of 96)
YB = 8           # image rows per block
NCORES = 8
HW = H * W

# (stationary_base, moving_base, moving_width, stationary_width, qcol_base)
CHUNKS = [(0, 0, 191, 128, 0), (128, 128, 191, 128, 191), (256, 256, 127, 64, 382)]

# tuning knobs (overridable before build())
CFG = dict(qd_eng="sync", load_eng="sync", diag_eng="sync", out_eng="sync",
           qs_bufs=4, psq_bufs=4, g_bufs=3, fs_bufs=3, qd_bufs=3, lt_bufs=4, rt_bufs=4)

def _eng(nc, name):
    return {"sync": nc.sync, "scalar": nc.scalar, "gpsimd": nc.gpsimd}[name]

def QD_ENG(nc): return _eng(nc, CFG["qd_eng"])
def LOAD_ENG(nc): return _eng(nc, CFG["load_eng"])
def DIAG_ENG(nc): return _eng(nc, CFG["diag_eng"])
def OUT_ENG(nc): return _eng(nc, CFG["out_eng"])


def build():
    nc = bacc.Bacc("TRN2", target_bir_lowering=False, debug=False,
                   enable_asserts=False, num_devices=NCORES)
    left = nc.dram_tensor("left", [C, H, W], F32, kind="ExternalInput").ap()
    right = nc.dram_tensor("right", [C, H, W], F32, kind="ExternalInput").ap()
    ident_d = nc.dram_tensor("ident", [128, 128], F32, kind="ExternalInput").ap()
    out = nc.dram_tensor("out", [D, H, W], F32, kind="ExternalOutput").ap()

    with ExitStack() as ctx:
        tc = ctx.enter_context(tile.TileContext(nc))
        const_pool = ctx.enter_context(tc.tile_pool(name="const", bufs=1))
        lt_pool = ctx.enter_context(tc.tile_pool(name="lt", bufs=CFG["lt_bufs"]))
        rt_pool = ctx.enter_context(tc.tile_pool(name="rt", bufs=CFG["rt_bufs"]))
        qs_pool = ctx.enter_context(tc.tile_pool(name="qs", bufs=CFG["qs_bufs"]))
        g_pool = ctx.enter_context(tc.tile_pool(name="g", bufs=CFG["g_bufs"]))
        fs_pool = ctx.enter_context(tc.tile_pool(name="fs", bufs=CFG["fs_bufs"]))
        psq_pool = ctx.enter_context(tc.tile_pool(name="psq", bufs=CFG["psq_bufs"], space="PSUM"))
        psf_pool = ctx.enter_context(tc.tile_pool(name="psf", bufs=2, space="PSUM"))
        qd_pool = ctx.enter_context(tc.tile_pool(name="qd", bufs=CFG["qd_bufs"], space="DRAM"))

        ident = const_pool.tile([128, 128], F32, tag="ident")
        nc.sync.dma_start(ident[:], ident_d[:])

        # Pre-zero every L-tile slot once: loads only write cols [0,320); the
        # pad [320,384) must read as zero for the last chunk's band tail.
        for _ in range(CFG["lt_bufs"]):
            z = lt_pool.tile([128, YB, 384], F32, tag="lt")
            nc.gpsimd.memset(z[:], 0.0)

        for yb in range(H // YB):
            y0 = yb * YB
            lts, rts = [], []
            for h in range(2):
                lt = lt_pool.tile([128, YB, 384], F32, tag="lt")
                LOAD_ENG(nc).dma_start(lt[:, :, 0:W], left[h * 128:(h + 1) * 128, y0:y0 + YB, :])
                lts.append(lt)
                rt = rt_pool.tile([128, YB, W], F32, tag="rt")
                LOAD_ENG(nc).dma_start(rt[:], right[h * 128:(h + 1) * 128, y0:y0 + YB, :])
                rts.append(rt)

            qd_t = qd_pool.tile([YB, 128, P], F32, tag="qd")

            for y in range(YB):
                qrow = psq_pool.tile([128, 509], F32, tag="qrow")
                for (sb, mb, mw, sw, qb) in CHUNKS:
                    for h in range(2):
                        nc.tensor.matmul(
                            qrow[0:sw, qb:qb + mw],
                            lhsT=rts[h][:, y, sb:sb + sw],
                            rhs=lts[h][:, y, mb:mb + mw],
                            start=(h == 0), stop=(h == 1),
                        )
                qs = qs_pool.tile([128, P], F32, tag="qs")
                nc.vector.tensor_scalar_mul(qs[:, 0:509], qrow[:], 1.0 / C)
                QD_ENG(nc).dma_start(qd_t[y, :, 0:509], qs[:, 0:509])

            # Diagonal band extraction via DRAM-side strides.
            g1 = g_pool.tile([128, YB, 2, D], F32, tag="g1")
            for c in range(2):
                src1 = bass.AP(qd_t[:].tensor, qd_t[0, 0, 191 * c:191 * c + 1].offset,
                               [[P + 1, 128], [128 * P, YB], [1, D]])
                dst1 = bass.AP(g1[:].tensor, g1[0, 0, c, 0:1].offset,
                               [[YB * 2 * D, 128], [2 * D, YB], [1, D]])
                DIAG_ENG(nc).dma_start(dst1, src1)
            g2 = g_pool.tile([64, YB, D], F32, tag="g2")
            src2 = bass.AP(qd_t[:].tensor, qd_t[0, 0, 382:383].offset,
                           [[P + 1, 64], [128 * P, YB], [1, D]])
            DIAG_ENG(nc).dma_start(g2[:], src2)

            fs1 = fs_pool.tile([128, YB, 128], F32, tag="fs1")
            fs2 = fs_pool.tile([64, YB, D], F32, tag="fs2")
            for y in range(YB):
                f1 = psf_pool.tile([128, 128], F32, tag="f1")
                nc.tensor.transpose(f1[:], g1[:, y, :, :], ident[:])
                nc.scalar.mul(fs1[:, y, :], f1[:], 1.0)
                f2 = psf_pool.tile([64, 64], F32, tag="f2")
                nc.tensor.transpose(f2[:], g2[:, y, :], ident[0:64, 0:64])
                nc.scalar.mul(fs2[:, y, :], f2[:], 1.0)

            # fs1 partitions p = 64*chunk + d hold cost[d, y, 128*chunk + i].
            row1 = YB * 128
            for c in range(2):
                src_o1 = bass.AP(fs1[:].tensor, fs1[64 * c, 0, 0:1].offset,
                                 [[row1, D], [128, YB], [1, 128]])
                dst_o1 = bass.AP(out.tensor, y0 * W + 128 * c,
                                 [[HW, D], [W, YB], [1, 128]])
                OUT_ENG(nc).dma_start(dst_o1, src_o1)
            row2 = YB * D
            src_o2 = bass.AP(fs2[:].tensor, fs2[:].offset,
                             [[row2, D], [D, YB], [1, D]])
            dst_o2 = bass.AP(out.tensor, y0 * W + 256,
                             [[HW, D], [W, YB], [1, D]])
            OUT_ENG(nc).dma_start(dst_o2, src_o2)

    nc.compile()
    return nc


_cache = {}


def _get_nc():
    if "nc" not in _cache:
        _cache["nc"] = build()
    return _cache["nc"]


def kernel(left_feature, right_feature, _trace=False):
    nc = _get_nc()
    eye = np.eye(128, dtype=np.float32)
    in_maps = [
        {"left": np.ascontiguousarray(left_feature[b]),
         "right": np.ascontiguousarray(right_feature[b]),
         "ident": eye}
        for b in range(B)
    ]
    res = run_bass_kernel_spmd(nc, in_maps, list(range(NCORES)), trace=_trace)
    out = np.stack([res.results[b]["out"] for b in range(B)]).astype(np.float32)
    if _trace:
        _cache["last_exec_time_ns"] = res.exec_time_ns
        _cache["last_res"] = res
    return out
